# revision 60
# baseline (speedup 1.0000x reference)
"""Trainium2 Bass kernel for NodeFeatureExtractor.

Key idea: bilinear sampling is linear and so is the first MLP layer, so
they commute.  Each core precomputes H = map @ W1_mapT once (16384 px x
128 out, fp16) on the PE, then per node gathers only the 4 corner rows
of H (4 x 128 fp16 = 1KB) instead of 4 x 512 fp32 map channels (8KB).
The coords features (cx, cy) are linear in pixel position, so they fold
exactly into H as two constant map channels; only [degree, dist] remain
as a rank-2 matmul folded into the same PSUM accumulation that also
performs the interp combine + transpose (matmul against identity).

H2 DRAM row p holds [H[p], H[p+FW]] so ONE 1KB SWDGE descriptor per
node fetches all 4 corners (gathers batched 1024 idx/instruction, the
HW ucode limit).  Interp products are split DVE (scalar_tensor_tensor
chains) / Act (scaled copies); their sums are free via PSUM-accumulated
transpose-matmuls against identity, sharing the bank with the rank-2
[deg, dist] extras matmul.  Degree normalization (global max) is folded
into the extras weight on the host (counts come from host bincount
anyway — a device AllReduce acts as a global barrier in Tile and costs
~60us).  Stage 2 (ReLU, MLP2, output) of chunk c-1 is emitted after
stage 1 of chunk c so the in-order Act/PE queues pipeline cleanly.
Output is written channel-major [128, ns] fp16; host transposes.
"""
import threading
from contextlib import ExitStack

import numpy as np

import bass_rust
import concourse.bass as bass
import concourse.bacc as bacc
import concourse.mybir as mybir
import concourse.tile as tile
from concourse import masks

F32 = mybir.dt.float32
F16 = mybir.dt.float16
I32 = mybir.dt.int32
I16 = mybir.dt.int16
ALU = mybir.AluOpType
ACTF = mybir.ActivationFunctionType
AX = mybir.AxisListType

N_NODES = 200000
N_CORES = 8
HID = 128
FH = FW = 128
NPIX = FH * FW          # 16384
MCH = 512               # padded map channels (480 bb + 4 seg + 2 coord + 26 z)
NCH = 512               # nodes per compute chunk
GCH = 1024              # nodes per gather instruction (HW limit < 2048)


class CFG:
    def __init__(self, n_shard, n_cores, image_size=512.0):
        assert n_shard % NCH == 0
        self.n_shard = n_shard                      # nodes per core (padded)
        self.n_cores = n_cores
        self.pad_n = n_shard * n_cores              # padded total nodes
        self.image_size = float(image_size)


def build_nc(cfg: CFG) -> bass.Bass:
    nc = bacc.Bacc("TRN2", num_devices=cfg.n_cores)
    ns, npc = cfg.n_shard, cfg.n_shard // 128      # node cols (p-major)
    nwc = cfg.n_shard // 16                        # node cols (16-wrap)
    n_chunks = ns // NCH
    sx = (FW - 1) / cfg.image_size                 # pixel scale

    map_cm = nc.dram_tensor("map_cm", [MCH, NPIX], F16, kind="ExternalInput")
    verts_w = nc.dram_tensor("verts_w", [2, 128, nwc], F32, kind="ExternalInput")
    verts_c = nc.dram_tensor("verts_c", [128, npc, 2], F32, kind="ExternalInput")
    extras = nc.dram_tensor("extras", [2, ns], F16, kind="ExternalInput")
    w1aT = nc.dram_tensor("w1aT", [4, 128, 128], F16, kind="ExternalInput")
    w1x = nc.dram_tensor("w1x", [2, 128], F16, kind="ExternalInput")
    w2T = nc.dram_tensor("w2T", [128, 128], F16, kind="ExternalInput")
    b1 = nc.dram_tensor("b1", [128, 1], F32, kind="ExternalInput")
    b2 = nc.dram_tensor("b2", [128, 1], F32, kind="ExternalInput")
    h_out = nc.dram_tensor("h_out", [128, ns], F16, kind="ExternalOutput")
    # H2 row p holds [H[p], H[p+FW]]: one 1KB gather descriptor covering
    # rows p..p+1 delivers all 4 bilinear corners of a node
    H2 = nc.dram_tensor("H2", [NPIX, 2 * HID], F16, kind="Internal")
    gsrc = bass_rust.AP(H2[:, :].tensor, 0,
                        [[2 * HID, NPIX - 2], [1, 4 * HID]])

    with tile.TileContext(nc) as tc, ExitStack() as ctx:

        st = ctx.enter_context(tc.tile_pool(name="static", bufs=1))
        dram = ctx.enter_context(tc.tile_pool(name="dram", bufs=1, space="DRAM"))
        mpool = ctx.enter_context(tc.tile_pool(name="mapp", bufs=4))
        hopool = ctx.enter_context(tc.tile_pool(name="hop", bufs=3))
        hspool = ctx.enter_context(tc.tile_pool(name="hsp", bufs=3))
        gpool = ctx.enter_context(tc.tile_pool(name="gather", bufs=4))
        tpool = ctx.enter_context(tc.tile_pool(name="tmps", bufs=2))
        h1pool = ctx.enter_context(tc.tile_pool(name="h1p", bufs=2))
        opool = ctx.enter_context(tc.tile_pool(name="outs", bufs=2))


        ident = st.tile([128, 128], F16)
        masks.make_identity(nc, ident[:])

        # ---- static loads
        w1a_sb = st.tile([128, 4, 128], F16)
        nc.sync.dma_start(w1a_sb[:], w1aT[:, :, :].rearrange("k p m -> p k m"))
        w1x_sb = st.tile([2, 128], F16)
        nc.sync.dma_start(w1x_sb[:], w1x[:, :])
        w2_sb = st.tile([128, 128], F16)
        nc.sync.dma_start(w2_sb[:], w2T[:, :])
        b1_sb = st.tile([128, 1], F32)
        nc.sync.dma_start(b1_sb[:], b1[:, :])
        b2_sb = st.tile([128, 1], F32)
        nc.sync.dma_start(b2_sb[:], b2[:, :])
        ex_sb = st.tile([2, ns], F16)
        nc.sync.dma_start(ex_sb[:], extras[:, :])

        # ---- batched gather-index computation (16-wrap layout)
        vw = st.tile([128, 2, nwc], F32)
        nc.sync.dma_start(vw[:], verts_w[:, :, :].rearrange("d p c -> p d c"))
        fx = st.tile([128, nwc], F32)
        fy = st.tile([128, nwc], F32)
        ti = st.tile([128, nwc], I32)
        tf = st.tile([128, nwc], F32)
        ti2 = st.tile([128, nwc], I32)
        tf2 = st.tile([128, nwc], F32)

        def floor_ip(eng, x, i_t, f_t):
            # x <- floor(x), robust to cast rounding mode (x >= 0)
            eng.tensor_copy(i_t[:], x)
            eng.tensor_copy(f_t[:], i_t[:])
            eng.tensor_tensor(x, f_t[:], x, ALU.is_gt)   # x = (f > x)
            eng.tensor_tensor(x, f_t[:], x, ALU.subtract)

        nc.vector.tensor_scalar(fx[:], vw[:, 0, :], sx, None, ALU.mult)
        floor_ip(nc.vector, fx[:], ti, tf)
        nc.vector.tensor_scalar(fy[:], vw[:, 1, :], sx, None, ALU.mult)
        floor_ip(nc.vector, fy[:], ti2, tf2)
        nc.vector.scalar_tensor_tensor(fx[:], fy[:], float(FW), fx[:],
                                       ALU.mult, ALU.add)
        idx16 = st.tile([128, nwc], I16)
        nc.vector.tensor_copy(idx16[:], fx[:])

        # ---- per-node bilinear weights (p-major layout)
        vc = st.tile([128, npc, 2], F32)
        nc.sync.dma_start(vc[:], verts_c[:, :, :])
        wx = st.tile([128, npc], F32)
        wy = st.tile([128, npc], F32)
        wti = st.tile([128, npc], I32)
        wtf = st.tile([128, npc], F32)
        wti2 = st.tile([128, npc], I32)
        wtf2 = st.tile([128, npc], F32)
        nc.vector.tensor_scalar(wx[:], vc[:, :, 0], sx, None, ALU.mult)
        nc.vector.tensor_scalar(wy[:], vc[:, :, 1], sx, None, ALU.mult)

        def frac_ip(eng, x, i_t, f_t):
            # x <- x - floor(x), robust to cast rounding mode (x >= 0)
            eng.tensor_copy(i_t[:], x)
            eng.tensor_copy(f_t[:], i_t[:])
            eng.tensor_tensor(i_t[:].bitcast(F32), f_t[:], x, ALU.is_gt)
            eng.tensor_tensor(f_t[:], f_t[:], i_t[:].bitcast(F32),
                              ALU.subtract)
            eng.tensor_tensor(x, x, f_t[:], ALU.subtract)

        frac_ip(nc.vector, wx[:], wti, wtf)
        frac_ip(nc.vector, wy[:], wti2, wtf2)
        w11 = st.tile([128, npc], F32)
        nc.vector.tensor_tensor(w11[:], wx[:], wy[:], ALU.mult)
        w01 = st.tile([128, npc], F32)
        nc.vector.tensor_tensor(w01[:], wx[:], w11[:], ALU.subtract)
        w10 = st.tile([128, npc], F32)
        nc.vector.tensor_tensor(w10[:], wy[:], w11[:], ALU.subtract)
        w00 = st.tile([128, npc], F32)
        nc.vector.tensor_scalar(wx[:], wx[:], -1.0, 1.0, ALU.mult, ALU.add)
        nc.vector.tensor_tensor(w00[:], wx[:], w10[:], ALU.subtract)


        # ---- precompute H = map_cm.T @ w1a  (pixel-major fp16 in DRAM)
        zpad = st.tile([128, 128], F16)
        nc.vector.memset(zpad[:], 0.0)
        # last FW rows of the [H[p+FW]] half have no source; zero them
        # (never gathered: idx+1 <= NPIX-FW-1)
        nc.sync.dma_start(H2[NPIX - FW:NPIX, HID:2 * HID], zpad[:])
        with tc.tile_pool(name="ps_pre", bufs=2, space="PSUM") as prep, \
                tc.tile_pool(name="ps_pT", bufs=2, space="PSUM") as prepT:
            for t in range(NPIX // 512):
                mt = mpool.tile([128, 4, 512], F16)
                nc.sync.dma_start(
                    mt[:], map_cm[:, 512 * t:512 * (t + 1)]
                    .rearrange("(k p) x -> p k x", p=128))
                # stationary = w1a (4 ldweights) -> psum is [out, px]
                pho = prep.tile([128, 512], F32, tag="pho")
                for k in range(4):
                    nc.tensor.matmul(pho[:, :], w1a_sb[:, k, :], mt[:, k, :],
                                     start=(k == 0), stop=(k == 3))
                hso = hopool.tile([128, 512], F16, tag="hso")
                nc.scalar.activation(hso[:], pho[:], ACTF.Copy)
                # transpose to pixel-major [px, out] on PE
                ph = prepT.tile([128, 4, 128], F32, tag="phT")
                for sub in range(4):
                    # only the first write may start (zeroes the whole bank)
                    nc.tensor.matmul(ph[:, sub, :],
                                     hso[:, 128 * sub:128 * (sub + 1)],
                                     ident[:], start=(sub == 0),
                                     stop=(sub == 3), skip_group_check=True)
                hs = hspool.tile([128, 4, 128], F16, tag="hs")
                nc.vector.tensor_copy(hs[:], ph[:])
                nc.sync.dma_start(
                    H2[512 * t:512 * (t + 1), 0:HID]
                    .rearrange("(s p) h -> p s h", p=128), hs[:])
                # second copy shifted FW rows up fills the [H[p+FW]] half
                if t == 0:
                    nc.sync.dma_start(
                        H2[0:384, HID:2 * HID]
                        .rearrange("(s p) h -> p s h", p=128), hs[:, 1:4, :])
                else:
                    nc.sync.dma_start(
                        H2[512 * t - FW:512 * (t + 1) - FW, HID:2 * HID]
                        .rearrange("(s p) h -> p s h", p=128), hs[:])

        # ---- main loop: gather H corners, interp, extras, ReLU, MLP2.
        # Software-pipelined: stage 2 of chunk c-1 (h1/mm2/h2/out) is
        # emitted after stage 1 of chunk c, so the in-order Act and PE
        # queues never stall on the previous chunk's tail.
        pstp = ctx.enter_context(tc.tile_pool(name="ps_t", bufs=4,
                                              space="PSUM"))
        ps2p = ctx.enter_context(tc.tile_pool(name="ps_2", bufs=3,
                                              space="PSUM"))
        def stage2(pst_p, c0_p):
            h1 = h1pool.tile([128, 512], F16, tag="h1")
            nc.scalar.activation(h1[:], pst_p[:], ACTF.Relu,
                                 bias=b1_sb[:, :])
            ps2 = ps2p.tile([128, 512], F32, tag="ps2")
            nc.tensor.matmul(ps2[:], w2_sb[:], h1[:], start=True,
                             stop=True)
            h2 = opool.tile([128, 512], F16, tag="h2")
            nc.scalar.activation(h2[:], ps2[:], ACTF.Relu,
                                 bias=b2_sb[:, :])
            nc.sync.dma_start(h_out[:, c0_p:c0_p + NCH], h2[:])

        pending = None
        for gs in range(0, ns, GCH):
            gn = min(GCH, ns - gs)
            ng = gn // 128
            g0t = gpool.tile([128, GCH // 128, 512], F16, tag="g0")
            nc.gpsimd.dma_gather(g0t[:, 0:ng, :], gsrc,
                                 idx16[:, gs // 16:(gs + gn) // 16],
                                 gn, gn, 4 * HID, elem_step=2 * HID)
            for c0 in range(gs, gs + gn, NCH):
                pst = pstp.tile([128, 512], F32, tag="pst")
                # start=True zeroes the whole 2KB PSUM bank: the bank-wide
                # extras matmul must come first, transposes accumulate after
                nc.tensor.matmul(pst[:, :], w1x_sb[:, :],
                                 ex_sb[:, c0:c0 + NCH], start=True,
                                 stop=False, skip_group_check=True)
                for g in range(4):
                    j = (c0 - gs) // 128 + g
                    col = c0 // 128 + g
                    # gathered node row: [C00, C10, C01, C11] (x-pairs of
                    # H2 rows idx, idx+1)
                    t1 = tpool.tile([128, 128], F16, tag=f"t{g}a")
                    nc.vector.tensor_scalar(
                        t1[:], g0t[:, j, 0:128], w00[:, col:col + 1],
                        None, ALU.mult)
                    nc.vector.scalar_tensor_tensor(
                        t1[:], g0t[:, j, 256:384], w01[:, col:col + 1],
                        t1[:], ALU.mult, ALU.add)
                    nc.tensor.matmul(pst[:, 128 * g:128 * (g + 1)], t1[:],
                                     ident[:], start=False, stop=False,
                                     skip_group_check=True)
                    if g == 3:
                        # last group's chainB fused on DVE (one transpose)
                        t2 = tpool.tile([128, 128], F16, tag=f"t{g}b")
                        nc.vector.tensor_scalar(
                            t2[:], g0t[:, j, 128:256], w10[:, col:col + 1],
                            None, ALU.mult)
                        nc.vector.scalar_tensor_tensor(
                            t2[:], g0t[:, j, 384:512], w11[:, col:col + 1],
                            t2[:], ALU.mult, ALU.add)
                        nc.tensor.matmul(pst[:, 128 * g:128 * (g + 1)],
                                         t2[:], ident[:], start=False,
                                         stop=True, skip_group_check=True)
                    else:
                        # chainB on Act (scaled copies); adds free in PSUM
                        t2 = tpool.tile([128, 128], F16, tag=f"t{g}b")
                        nc.scalar.activation(t2[:], g0t[:, j, 128:256],
                                             ACTF.Copy,
                                             scale=w10[:, col:col + 1])
                        t2b = tpool.tile([128, 128], F16, tag=f"t{g}c")
                        nc.scalar.activation(t2b[:], g0t[:, j, 384:512],
                                             ACTF.Copy,
                                             scale=w11[:, col:col + 1])
                        nc.tensor.matmul(pst[:, 128 * g:128 * (g + 1)],
                                         t2[:], ident[:], start=False,
                                         stop=False, skip_group_check=True)
                        nc.tensor.matmul(pst[:, 128 * g:128 * (g + 1)],
                                         t2b[:], ident[:], start=False,
                                         stop=False, skip_group_check=True)
                if pending is not None:
                    stage2(*pending)
                pending = (pst, c0)
        stage2(*pending)

    nc.compile()
    return nc


# ---------------- host side ----------------

def prep_inputs(cfg: CFG, vertices, backbone_features, seg_probs, edge_index,
                W1, W2):
    """Host prep: layout transforms + exact integer degree counts."""
    im = cfg.image_size
    v = np.asarray(vertices, np.float32)
    n = v.shape[0]
    if n < cfg.pad_n:
        v = np.concatenate([v, np.repeat(v[-1:], cfg.pad_n - n, 0)], 0)
    ep = np.asarray(edge_index).reshape(-1).astype(np.int64)
    degree = np.bincount(ep, minlength=cfg.pad_n).astype(np.float32)

    m = np.zeros((MCH, NPIX), np.float16)
    m[:480] = np.asarray(backbone_features, np.float32).reshape(480, -1)
    m[480:484] = np.asarray(seg_probs, np.float32).reshape(4, -1)
    # coords features are linear in pixel position: fold into the map
    px = np.arange(FW, dtype=np.float32) / (FW - 1)
    m[484] = np.tile(px, FH)                       # cx = ix/(W-1)
    m[485] = np.repeat(px, FW)                     # cy = iy/(H-1)

    W1 = np.asarray(W1, np.float32)
    w1a = np.zeros((MCH, 128), np.float32)
    w1a[0:480] = W1[:, 2:482].T
    w1a[480:484] = W1[:, 482:486].T
    w1a[484] = W1[:, 0]
    w1a[485] = W1[:, 1]
    w1aT = np.ascontiguousarray(w1a.reshape(4, 128, 128)).astype(np.float16)
    # degree normalization (global max) folded into the deg weight row
    w1x = np.ascontiguousarray(W1[:, 486:488].T)  # deg, dist
    w1x[0] /= degree.max() + 1e-6
    w1x = w1x.astype(np.float16)
    w2T = np.ascontiguousarray(np.asarray(W2, np.float32).T).astype(np.float16)

    # dist-to-boundary feature (host: pure elementwise data prep)
    dist = np.minimum(np.minimum(v[:, 0], im - v[:, 0]),
                      np.minimum(v[:, 1], im - v[:, 1])) / (im / 2)

    in_maps = []
    for c in range(cfg.n_cores):
        lo, hi = c * cfg.n_shard, (c + 1) * cfg.n_shard
        vcs = v[lo:hi]
        verts_w = np.ascontiguousarray(
            vcs.reshape(-1, 16, 2).transpose(2, 1, 0))       # (2,16,nwc)
        verts_w = np.ascontiguousarray(np.tile(verts_w, (1, 8, 1)))
        verts_c = np.ascontiguousarray(
            vcs.reshape(-1, 128, 2).transpose(1, 0, 2))      # (128,npc,2)
        extras = np.stack([degree[lo:hi], dist[lo:hi]]).astype(np.float16)
        in_maps.append({
            "map_cm": m, "verts_w": verts_w, "verts_c": verts_c,
            "extras": extras, "w1aT": w1aT, "w1x": w1x, "w2T": w2T,
        })
    return in_maps


_NC_CACHE: dict = {}
_NC_LOCK = threading.Lock()


def kernel(vertices, backbone_features, seg_probs, edge_index, W1, b1, W2, b2,
           image_size):
    from concourse.bass_utils import run_bass_kernel_spmd

    n = int(np.asarray(vertices).shape[0])
    n_shard = -(-n // (N_CORES * NCH)) * NCH
    cfg = CFG(n_shard, N_CORES, float(np.asarray(image_size)))

    key = (cfg.n_shard, cfg.n_cores, cfg.image_size)
    with _NC_LOCK:
        if key not in _NC_CACHE:
            _NC_CACHE[key] = build_nc(cfg)
        nc = _NC_CACHE[key]

    in_maps = prep_inputs(cfg, vertices, backbone_features, seg_probs,
                          edge_index, W1, W2)
    b1c = np.ascontiguousarray(np.asarray(b1, np.float32).reshape(128, 1))
    b2c = np.ascontiguousarray(np.asarray(b2, np.float32).reshape(128, 1))
    for im in in_maps:
        im["b1"] = b1c
        im["b2"] = b2c

    res = run_bass_kernel_spmd(nc, in_maps, core_ids=list(range(N_CORES)))
    h = np.concatenate(
        [res.results[c]["h_out"].T for c in range(N_CORES)], 0)
    return np.ascontiguousarray(h[:n]).astype(np.float32)


# revision 61
# speedup vs baseline: 1.0007x; 1.0007x over previous
"""Trainium2 Bass kernel for NodeFeatureExtractor.

Key idea: bilinear sampling is linear and so is the first MLP layer, so
they commute.  Each core precomputes H = map @ W1_mapT once (16384 px x
128 out, fp16) on the PE, then per node gathers only the 4 corner rows
of H (4 x 128 fp16 = 1KB) instead of 4 x 512 fp32 map channels (8KB).
The coords features (cx, cy) are linear in pixel position, so they fold
exactly into H as two constant map channels; only [degree, dist] remain
as a rank-2 matmul folded into the same PSUM accumulation that also
performs the interp combine + transpose (matmul against identity).

H2 DRAM row p holds [H[p], H[p+FW]] so ONE 1KB SWDGE descriptor per
node fetches all 4 corners (gathers batched 1024 idx/instruction, the
HW ucode limit).  Interp products are split DVE (scalar_tensor_tensor
chains) / Act (scaled copies); their sums are free via PSUM-accumulated
transpose-matmuls against identity, sharing the bank with the rank-2
[deg, dist] extras matmul.  Degree normalization (global max) is folded
into the extras weight on the host (counts come from host bincount
anyway — a device AllReduce acts as a global barrier in Tile and costs
~60us).  Stage 2 (ReLU, MLP2, output) of chunk c-1 is emitted after
stage 1 of chunk c so the in-order Act/PE queues pipeline cleanly.
Output is written channel-major [128, ns] fp16; host transposes.
"""
import threading
from contextlib import ExitStack

import numpy as np

import bass_rust
import concourse.bass as bass
import concourse.bacc as bacc
import concourse.mybir as mybir
import concourse.tile as tile
from concourse import masks

F32 = mybir.dt.float32
F16 = mybir.dt.float16
I32 = mybir.dt.int32
I16 = mybir.dt.int16
ALU = mybir.AluOpType
ACTF = mybir.ActivationFunctionType
AX = mybir.AxisListType

N_NODES = 200000
N_CORES = 8
HID = 128
FH = FW = 128
NPIX = FH * FW          # 16384
MCH = 512               # padded map channels (480 bb + 4 seg + 2 coord + 26 z)
NCH = 512               # nodes per compute chunk
GCH = 1024              # nodes per gather instruction (HW limit < 2048)


class CFG:
    def __init__(self, n_shard, n_cores, image_size=512.0):
        assert n_shard % NCH == 0
        self.n_shard = n_shard                      # nodes per core (padded)
        self.n_cores = n_cores
        self.pad_n = n_shard * n_cores              # padded total nodes
        self.image_size = float(image_size)


def build_nc(cfg: CFG) -> bass.Bass:
    nc = bacc.Bacc("TRN2", num_devices=cfg.n_cores)
    ns, npc = cfg.n_shard, cfg.n_shard // 128      # node cols (p-major)
    nwc = cfg.n_shard // 16                        # node cols (16-wrap)
    n_chunks = ns // NCH
    sx = (FW - 1) / cfg.image_size                 # pixel scale

    map_cm = nc.dram_tensor("map_cm", [MCH, NPIX], F16, kind="ExternalInput")
    verts_w = nc.dram_tensor("verts_w", [2, 128, nwc], F32, kind="ExternalInput")
    verts_c = nc.dram_tensor("verts_c", [128, npc, 2], F32, kind="ExternalInput")
    extras = nc.dram_tensor("extras", [2, ns], F16, kind="ExternalInput")
    w1aT = nc.dram_tensor("w1aT", [4, 128, 128], F16, kind="ExternalInput")
    w1x = nc.dram_tensor("w1x", [2, 128], F16, kind="ExternalInput")
    w2T = nc.dram_tensor("w2T", [128, 128], F16, kind="ExternalInput")
    b1 = nc.dram_tensor("b1", [128, 1], F32, kind="ExternalInput")
    b2 = nc.dram_tensor("b2", [128, 1], F32, kind="ExternalInput")
    h_out = nc.dram_tensor("h_out", [128, ns], F16, kind="ExternalOutput")
    # H2 row p holds [H[p], H[p+FW]]: one 1KB gather descriptor covering
    # rows p..p+1 delivers all 4 bilinear corners of a node
    H2 = nc.dram_tensor("H2", [NPIX, 2 * HID], F16, kind="Internal")
    gsrc = bass_rust.AP(H2[:, :].tensor, 0,
                        [[2 * HID, NPIX - 2], [1, 4 * HID]])

    with tile.TileContext(nc) as tc, ExitStack() as ctx:

        st = ctx.enter_context(tc.tile_pool(name="static", bufs=1))
        dram = ctx.enter_context(tc.tile_pool(name="dram", bufs=1, space="DRAM"))
        mpool = ctx.enter_context(tc.tile_pool(name="mapp", bufs=4))
        hopool = ctx.enter_context(tc.tile_pool(name="hop", bufs=3))
        hspool = ctx.enter_context(tc.tile_pool(name="hsp", bufs=3))
        gpool = ctx.enter_context(tc.tile_pool(name="gather", bufs=4))
        tpool = ctx.enter_context(tc.tile_pool(name="tmps", bufs=2))
        h1pool = ctx.enter_context(tc.tile_pool(name="h1p", bufs=2))
        opool = ctx.enter_context(tc.tile_pool(name="outs", bufs=2))


        ident = st.tile([128, 128], F16)
        masks.make_identity(nc, ident[:])

        # ---- static loads
        w1a_sb = st.tile([128, 4, 128], F16)
        nc.sync.dma_start(w1a_sb[:], w1aT[:, :, :].rearrange("k p m -> p k m"))
        w1x_sb = st.tile([2, 128], F16)
        nc.sync.dma_start(w1x_sb[:], w1x[:, :])
        w2_sb = st.tile([128, 128], F16)
        nc.sync.dma_start(w2_sb[:], w2T[:, :])
        b1_sb = st.tile([128, 1], F32)
        nc.sync.dma_start(b1_sb[:], b1[:, :])
        b2_sb = st.tile([128, 1], F32)
        nc.sync.dma_start(b2_sb[:], b2[:, :])
        ex_sb = st.tile([2, ns], F16)
        nc.sync.dma_start(ex_sb[:], extras[:, :])

        # ---- batched gather-index computation (16-wrap layout)
        vw = st.tile([128, 2, nwc], F32)
        nc.sync.dma_start(vw[:], verts_w[:, :, :].rearrange("d p c -> p d c"))
        fx = st.tile([128, nwc], F32)
        fy = st.tile([128, nwc], F32)
        ti = st.tile([128, nwc], I32)
        tf = st.tile([128, nwc], F32)
        ti2 = st.tile([128, nwc], I32)
        tf2 = st.tile([128, nwc], F32)

        def floor_ip(eng, x, i_t, f_t):
            # x <- floor(x), robust to cast rounding mode (x >= 0)
            eng.tensor_copy(i_t[:], x)
            eng.tensor_copy(f_t[:], i_t[:])
            eng.tensor_tensor(x, f_t[:], x, ALU.is_gt)   # x = (f > x)
            eng.tensor_tensor(x, f_t[:], x, ALU.subtract)

        nc.vector.tensor_scalar(fx[:], vw[:, 0, :], sx, None, ALU.mult)
        floor_ip(nc.vector, fx[:], ti, tf)
        nc.vector.tensor_scalar(fy[:], vw[:, 1, :], sx, None, ALU.mult)
        floor_ip(nc.vector, fy[:], ti2, tf2)
        nc.vector.scalar_tensor_tensor(fx[:], fy[:], float(FW), fx[:],
                                       ALU.mult, ALU.add)
        idx16 = st.tile([128, nwc], I16)
        nc.vector.tensor_copy(idx16[:], fx[:])

        # ---- per-node bilinear weights (p-major layout)
        vc = st.tile([128, npc, 2], F32)
        nc.sync.dma_start(vc[:], verts_c[:, :, :])
        wx = st.tile([128, npc], F32)
        wy = st.tile([128, npc], F32)
        wti = st.tile([128, npc], I32)
        wtf = st.tile([128, npc], F32)
        wti2 = st.tile([128, npc], I32)
        wtf2 = st.tile([128, npc], F32)
        nc.vector.tensor_scalar(wx[:], vc[:, :, 0], sx, None, ALU.mult)
        nc.vector.tensor_scalar(wy[:], vc[:, :, 1], sx, None, ALU.mult)

        def frac_ip(eng, x, i_t, f_t):
            # x <- x - floor(x), robust to cast rounding mode (x >= 0)
            eng.tensor_copy(i_t[:], x)
            eng.tensor_copy(f_t[:], i_t[:])
            eng.tensor_tensor(i_t[:].bitcast(F32), f_t[:], x, ALU.is_gt)
            eng.tensor_tensor(f_t[:], f_t[:], i_t[:].bitcast(F32),
                              ALU.subtract)
            eng.tensor_tensor(x, x, f_t[:], ALU.subtract)

        frac_ip(nc.vector, wx[:], wti, wtf)
        frac_ip(nc.vector, wy[:], wti2, wtf2)
        w11 = st.tile([128, npc], F32)
        nc.vector.tensor_tensor(w11[:], wx[:], wy[:], ALU.mult)
        w01 = st.tile([128, npc], F32)
        nc.vector.tensor_tensor(w01[:], wx[:], w11[:], ALU.subtract)
        w10 = st.tile([128, npc], F32)
        nc.vector.tensor_tensor(w10[:], wy[:], w11[:], ALU.subtract)
        w00 = st.tile([128, npc], F32)
        nc.vector.tensor_scalar(wx[:], wx[:], -1.0, 1.0, ALU.mult, ALU.add)
        nc.vector.tensor_tensor(w00[:], wx[:], w10[:], ALU.subtract)


        # ---- precompute H = map_cm.T @ w1a  (pixel-major fp16 in DRAM)
        zpad = st.tile([128, 128], F16)
        nc.vector.memset(zpad[:], 0.0)
        # last FW rows of the [H[p+FW]] half have no source; zero them
        # (never gathered: idx+1 <= NPIX-FW-1)
        nc.sync.dma_start(H2[NPIX - FW:NPIX, HID:2 * HID], zpad[:])
        with tc.tile_pool(name="ps_pre", bufs=2, space="PSUM") as prep, \
                tc.tile_pool(name="ps_pT", bufs=2, space="PSUM") as prepT:
            for t in range(NPIX // 512):
                mt = mpool.tile([128, 4, 512], F16)
                nc.sync.dma_start(
                    mt[:], map_cm[:, 512 * t:512 * (t + 1)]
                    .rearrange("(k p) x -> p k x", p=128))
                # stationary = w1a (4 ldweights) -> psum is [out, px]
                pho = prep.tile([128, 512], F32, tag="pho")
                for k in range(4):
                    nc.tensor.matmul(pho[:, :], w1a_sb[:, k, :], mt[:, k, :],
                                     start=(k == 0), stop=(k == 3))
                hso = hopool.tile([128, 512], F16, tag="hso")
                nc.scalar.activation(hso[:], pho[:], ACTF.Copy)
                # transpose to pixel-major [px, out] on PE
                ph = prepT.tile([128, 4, 128], F32, tag="phT")
                for sub in range(4):
                    # only the first write may start (zeroes the whole bank)
                    nc.tensor.matmul(ph[:, sub, :],
                                     hso[:, 128 * sub:128 * (sub + 1)],
                                     ident[:], start=(sub == 0),
                                     stop=(sub == 3), skip_group_check=True)
                hs = hspool.tile([128, 4, 128], F16, tag="hs")
                nc.vector.tensor_copy(hs[:], ph[:])
                nc.sync.dma_start(
                    H2[512 * t:512 * (t + 1), 0:HID]
                    .rearrange("(s p) h -> p s h", p=128), hs[:])
                # second copy shifted FW rows up fills the [H[p+FW]] half
                if t == 0:
                    nc.sync.dma_start(
                        H2[0:384, HID:2 * HID]
                        .rearrange("(s p) h -> p s h", p=128), hs[:, 1:4, :])
                else:
                    nc.sync.dma_start(
                        H2[512 * t - FW:512 * (t + 1) - FW, HID:2 * HID]
                        .rearrange("(s p) h -> p s h", p=128), hs[:])

        # ---- main loop: gather H corners, interp, extras, ReLU, MLP2.
        # Software-pipelined: stage 2 of chunk c-1 (h1/mm2/h2/out) is
        # emitted after stage 1 of chunk c, so the in-order Act and PE
        # queues never stall on the previous chunk's tail.
        pstp = ctx.enter_context(tc.tile_pool(name="ps_t", bufs=3,
                                              space="PSUM"))
        ps2p = ctx.enter_context(tc.tile_pool(name="ps_2", bufs=3,
                                              space="PSUM"))
        def stage2(pst_p, c0_p):
            h1 = h1pool.tile([128, 512], F16, tag="h1")
            nc.scalar.activation(h1[:], pst_p[:], ACTF.Relu,
                                 bias=b1_sb[:, :])
            ps2 = ps2p.tile([128, 512], F32, tag="ps2")
            nc.tensor.matmul(ps2[:], w2_sb[:], h1[:], start=True,
                             stop=True)
            h2 = opool.tile([128, 512], F16, tag="h2")
            nc.scalar.activation(h2[:], ps2[:], ACTF.Relu,
                                 bias=b2_sb[:, :])
            nc.sync.dma_start(h_out[:, c0_p:c0_p + NCH], h2[:])

        pending = None
        for gs in range(0, ns, GCH):
            gn = min(GCH, ns - gs)
            ng = gn // 128
            g0t = gpool.tile([128, GCH // 128, 512], F16, tag="g0")
            nc.gpsimd.dma_gather(g0t[:, 0:ng, :], gsrc,
                                 idx16[:, gs // 16:(gs + gn) // 16],
                                 gn, gn, 4 * HID, elem_step=2 * HID)
            for c0 in range(gs, gs + gn, NCH):
                pst = pstp.tile([128, 512], F32, tag="pst")
                # start=True zeroes the whole 2KB PSUM bank: the bank-wide
                # extras matmul must come first, transposes accumulate after
                nc.tensor.matmul(pst[:, :], w1x_sb[:, :],
                                 ex_sb[:, c0:c0 + NCH], start=True,
                                 stop=False, skip_group_check=True)
                for g in range(4):
                    j = (c0 - gs) // 128 + g
                    col = c0 // 128 + g
                    # gathered node row: [C00, C10, C01, C11] (x-pairs of
                    # H2 rows idx, idx+1)
                    t1 = tpool.tile([128, 128], F16, tag=f"t{g}a")
                    nc.vector.tensor_scalar(
                        t1[:], g0t[:, j, 0:128], w00[:, col:col + 1],
                        None, ALU.mult)
                    nc.vector.scalar_tensor_tensor(
                        t1[:], g0t[:, j, 256:384], w01[:, col:col + 1],
                        t1[:], ALU.mult, ALU.add)
                    nc.tensor.matmul(pst[:, 128 * g:128 * (g + 1)], t1[:],
                                     ident[:], start=False, stop=False,
                                     skip_group_check=True)
                    if g == 3:
                        # last group's chainB fused on DVE (one transpose)
                        t2 = tpool.tile([128, 128], F16, tag=f"t{g}b")
                        nc.vector.tensor_scalar(
                            t2[:], g0t[:, j, 128:256], w10[:, col:col + 1],
                            None, ALU.mult)
                        nc.vector.scalar_tensor_tensor(
                            t2[:], g0t[:, j, 384:512], w11[:, col:col + 1],
                            t2[:], ALU.mult, ALU.add)
                        nc.tensor.matmul(pst[:, 128 * g:128 * (g + 1)],
                                         t2[:], ident[:], start=False,
                                         stop=True, skip_group_check=True)
                    else:
                        # chainB on Act (scaled copies); adds free in PSUM
                        t2 = tpool.tile([128, 128], F16, tag=f"t{g}b")
                        nc.scalar.activation(t2[:], g0t[:, j, 128:256],
                                             ACTF.Copy,
                                             scale=w10[:, col:col + 1])
                        t2b = tpool.tile([128, 128], F16, tag=f"t{g}c")
                        nc.scalar.activation(t2b[:], g0t[:, j, 384:512],
                                             ACTF.Copy,
                                             scale=w11[:, col:col + 1])
                        nc.tensor.matmul(pst[:, 128 * g:128 * (g + 1)],
                                         t2[:], ident[:], start=False,
                                         stop=False, skip_group_check=True)
                        nc.tensor.matmul(pst[:, 128 * g:128 * (g + 1)],
                                         t2b[:], ident[:], start=False,
                                         stop=False, skip_group_check=True)
                if pending is not None:
                    stage2(*pending)
                pending = (pst, c0)
        stage2(*pending)

    nc.compile()
    return nc


# ---------------- host side ----------------

def prep_inputs(cfg: CFG, vertices, backbone_features, seg_probs, edge_index,
                W1, W2):
    """Host prep: layout transforms + exact integer degree counts."""
    im = cfg.image_size
    v = np.asarray(vertices, np.float32)
    n = v.shape[0]
    if n < cfg.pad_n:
        v = np.concatenate([v, np.repeat(v[-1:], cfg.pad_n - n, 0)], 0)
    ep = np.asarray(edge_index).reshape(-1).astype(np.int64)
    degree = np.bincount(ep, minlength=cfg.pad_n).astype(np.float32)

    m = np.zeros((MCH, NPIX), np.float16)
    m[:480] = np.asarray(backbone_features, np.float32).reshape(480, -1)
    m[480:484] = np.asarray(seg_probs, np.float32).reshape(4, -1)
    # coords features are linear in pixel position: fold into the map
    px = np.arange(FW, dtype=np.float32) / (FW - 1)
    m[484] = np.tile(px, FH)                       # cx = ix/(W-1)
    m[485] = np.repeat(px, FW)                     # cy = iy/(H-1)

    W1 = np.asarray(W1, np.float32)
    w1a = np.zeros((MCH, 128), np.float32)
    w1a[0:480] = W1[:, 2:482].T
    w1a[480:484] = W1[:, 482:486].T
    w1a[484] = W1[:, 0]
    w1a[485] = W1[:, 1]
    w1aT = np.ascontiguousarray(w1a.reshape(4, 128, 128)).astype(np.float16)
    # degree normalization (global max) folded into the deg weight row
    w1x = np.ascontiguousarray(W1[:, 486:488].T)  # deg, dist
    w1x[0] /= degree.max() + 1e-6
    w1x = w1x.astype(np.float16)
    w2T = np.ascontiguousarray(np.asarray(W2, np.float32).T).astype(np.float16)

    # dist-to-boundary feature (host: pure elementwise data prep)
    dist = np.minimum(np.minimum(v[:, 0], im - v[:, 0]),
                      np.minimum(v[:, 1], im - v[:, 1])) / (im / 2)

    in_maps = []
    for c in range(cfg.n_cores):
        lo, hi = c * cfg.n_shard, (c + 1) * cfg.n_shard
        vcs = v[lo:hi]
        verts_w = np.ascontiguousarray(
            vcs.reshape(-1, 16, 2).transpose(2, 1, 0))       # (2,16,nwc)
        verts_w = np.ascontiguousarray(np.tile(verts_w, (1, 8, 1)))
        verts_c = np.ascontiguousarray(
            vcs.reshape(-1, 128, 2).transpose(1, 0, 2))      # (128,npc,2)
        extras = np.stack([degree[lo:hi], dist[lo:hi]]).astype(np.float16)
        in_maps.append({
            "map_cm": m, "verts_w": verts_w, "verts_c": verts_c,
            "extras": extras, "w1aT": w1aT, "w1x": w1x, "w2T": w2T,
        })
    return in_maps


_NC_CACHE: dict = {}
_NC_LOCK = threading.Lock()


def kernel(vertices, backbone_features, seg_probs, edge_index, W1, b1, W2, b2,
           image_size):
    from concourse.bass_utils import run_bass_kernel_spmd

    n = int(np.asarray(vertices).shape[0])
    n_shard = -(-n // (N_CORES * NCH)) * NCH
    cfg = CFG(n_shard, N_CORES, float(np.asarray(image_size)))

    key = (cfg.n_shard, cfg.n_cores, cfg.image_size)
    with _NC_LOCK:
        if key not in _NC_CACHE:
            _NC_CACHE[key] = build_nc(cfg)
        nc = _NC_CACHE[key]

    in_maps = prep_inputs(cfg, vertices, backbone_features, seg_probs,
                          edge_index, W1, W2)
    b1c = np.ascontiguousarray(np.asarray(b1, np.float32).reshape(128, 1))
    b2c = np.ascontiguousarray(np.asarray(b2, np.float32).reshape(128, 1))
    for im in in_maps:
        im["b1"] = b1c
        im["b2"] = b2c

    res = run_bass_kernel_spmd(nc, in_maps, core_ids=list(range(N_CORES)))
    h = np.concatenate(
        [res.results[c]["h_out"].T for c in range(N_CORES)], 0)
    return np.ascontiguousarray(h[:n]).astype(np.float32)


# revision 62
# speedup vs baseline: 1.0509x; 1.0502x over previous
"""Trainium2 Bass kernel for NodeFeatureExtractor.

Key idea: bilinear sampling is linear and so is the first MLP layer, so
they commute.  Each core precomputes H = map @ W1_mapT once (16384 px x
128 out, fp16) on the PE, then per node gathers only the 4 corner rows
of H (4 x 128 fp16 = 1KB) instead of 4 x 512 fp32 map channels (8KB).
The coords features (cx, cy) are linear in pixel position, so they fold
exactly into H as two constant map channels; only [degree, dist] remain
as a rank-2 matmul folded into the same PSUM accumulation that also
performs the interp combine + transpose (matmul against identity).

H2 DRAM row p holds [H[p], H[p+FW]] so ONE 1KB SWDGE descriptor per
node fetches all 4 corners (gathers batched 1024 idx/instruction, the
HW ucode limit).  Interp products are split DVE (scalar_tensor_tensor
chains) / Act (scaled copies); their sums are free via PSUM-accumulated
transpose-matmuls against identity, sharing the bank with the rank-2
[deg, dist] extras matmul.  Degree normalization (global max) is folded
into the extras weight on the host (counts come from host bincount
anyway — a device AllReduce acts as a global barrier in Tile and costs
~60us).  Stage 2 (ReLU, MLP2, output) of chunk c-1 is emitted after
stage 1 of chunk c so the in-order Act/PE queues pipeline cleanly.
Output is written channel-major [128, ns] fp16; host transposes.
"""
import threading
from contextlib import ExitStack

import numpy as np

import bass_rust
import concourse.bass as bass
import concourse.bacc as bacc
import concourse.mybir as mybir
import concourse.tile as tile
from concourse import masks

F32 = mybir.dt.float32
F16 = mybir.dt.float16
I32 = mybir.dt.int32
I16 = mybir.dt.int16
ALU = mybir.AluOpType
ACTF = mybir.ActivationFunctionType
AX = mybir.AxisListType

N_NODES = 200000
N_CORES = 8
HID = 128
FH = FW = 128
NPIX = FH * FW          # 16384
MCH = 512               # padded map channels (480 bb + 4 seg + 2 coord + 26 z)
NCH = 512               # nodes per compute chunk
GCH = 1024              # nodes per gather instruction (HW limit < 2048)


class CFG:
    def __init__(self, n_shard, n_cores, image_size=512.0):
        assert n_shard % NCH == 0
        self.n_shard = n_shard                      # nodes per core (padded)
        self.n_cores = n_cores
        self.pad_n = n_shard * n_cores              # padded total nodes
        self.image_size = float(image_size)


def build_nc(cfg: CFG) -> bass.Bass:
    nc = bacc.Bacc("TRN2", num_devices=cfg.n_cores)
    ns, npc = cfg.n_shard, cfg.n_shard // 128      # node cols (p-major)
    nwc = cfg.n_shard // 16                        # node cols (16-wrap)
    n_chunks = ns // NCH
    sx = (FW - 1) / cfg.image_size                 # pixel scale

    map_cm = nc.dram_tensor("map_cm", [MCH, NPIX], F16, kind="ExternalInput")
    verts_w = nc.dram_tensor("verts_w", [2, 128, nwc], F32, kind="ExternalInput")
    verts_c = nc.dram_tensor("verts_c", [128, npc, 2], F32, kind="ExternalInput")
    extras = nc.dram_tensor("extras", [2, ns], F16, kind="ExternalInput")
    w1aT = nc.dram_tensor("w1aT", [4, 128, 128], F16, kind="ExternalInput")
    w1x = nc.dram_tensor("w1x", [2, 128], F16, kind="ExternalInput")
    w2T = nc.dram_tensor("w2T", [128, 128], F16, kind="ExternalInput")
    b1 = nc.dram_tensor("b1", [128, 1], F32, kind="ExternalInput")
    b2 = nc.dram_tensor("b2", [128, 1], F32, kind="ExternalInput")
    h_out = nc.dram_tensor("h_out", [128, ns], F16, kind="ExternalOutput")
    # H2 row p holds [H[p], H[p+FW]]: one 1KB gather descriptor covering
    # rows p..p+1 delivers all 4 bilinear corners of a node
    H2 = nc.dram_tensor("H2", [NPIX, 2 * HID], F16, kind="Internal")
    gsrc = bass_rust.AP(H2[:, :].tensor, 0,
                        [[2 * HID, NPIX - 2], [1, 4 * HID]])

    with tile.TileContext(nc) as tc, ExitStack() as ctx:

        st = ctx.enter_context(tc.tile_pool(name="static", bufs=1))
        dram = ctx.enter_context(tc.tile_pool(name="dram", bufs=1, space="DRAM"))
        mpool = ctx.enter_context(tc.tile_pool(name="mapp", bufs=4))
        hopool = ctx.enter_context(tc.tile_pool(name="hop", bufs=3))
        hspool = ctx.enter_context(tc.tile_pool(name="hsp", bufs=3))
        gpool = ctx.enter_context(tc.tile_pool(name="gather", bufs=4))
        tpool = ctx.enter_context(tc.tile_pool(name="tmps", bufs=2))
        h1pool = ctx.enter_context(tc.tile_pool(name="h1p", bufs=2))
        opool = ctx.enter_context(tc.tile_pool(name="outs", bufs=2))


        ident = st.tile([128, 128], F16)
        masks.make_identity(nc, ident[:])

        # ---- static loads
        w1a_sb = st.tile([128, 4, 128], F16)
        nc.sync.dma_start(w1a_sb[:], w1aT[:, :, :].rearrange("k p m -> p k m"))
        w1x_sb = st.tile([2, 128], F16)
        nc.sync.dma_start(w1x_sb[:], w1x[:, :])
        w2_sb = st.tile([128, 128], F16)
        nc.sync.dma_start(w2_sb[:], w2T[:, :])
        b1_sb = st.tile([128, 1], F32)
        nc.sync.dma_start(b1_sb[:], b1[:, :])
        b2_sb = st.tile([128, 1], F32)
        nc.sync.dma_start(b2_sb[:], b2[:, :])
        ex_sb = st.tile([2, ns], F16)
        nc.sync.dma_start(ex_sb[:], extras[:, :])

        # ---- batched gather-index computation (16-wrap layout)
        vw = st.tile([128, 2, nwc], F32)
        nc.sync.dma_start(vw[:], verts_w[:, :, :].rearrange("d p c -> p d c"))
        fx = st.tile([128, nwc], F32)
        fy = st.tile([128, nwc], F32)
        ti = st.tile([128, nwc], I32)
        tf = st.tile([128, nwc], F32)
        ti2 = st.tile([128, nwc], I32)
        tf2 = st.tile([128, nwc], F32)

        def floor_ip(eng, x, i_t, f_t):
            # x <- floor(x), robust to cast rounding mode (x >= 0)
            eng.tensor_copy(i_t[:], x)
            eng.tensor_copy(f_t[:], i_t[:])
            eng.tensor_tensor(x, f_t[:], x, ALU.is_gt)   # x = (f > x)
            eng.tensor_tensor(x, f_t[:], x, ALU.subtract)

        nc.vector.tensor_scalar(fx[:], vw[:, 0, :], sx, None, ALU.mult)
        floor_ip(nc.vector, fx[:], ti, tf)
        nc.vector.tensor_scalar(fy[:], vw[:, 1, :], sx, None, ALU.mult)
        floor_ip(nc.vector, fy[:], ti2, tf2)
        nc.vector.scalar_tensor_tensor(fx[:], fy[:], float(FW), fx[:],
                                       ALU.mult, ALU.add)
        idx16 = st.tile([128, nwc], I16)
        nc.vector.tensor_copy(idx16[:], fx[:])

        # ---- per-node bilinear weights (p-major layout)
        vc = st.tile([128, npc, 2], F32)
        nc.sync.dma_start(vc[:], verts_c[:, :, :])
        wx = st.tile([128, npc], F32)
        wy = st.tile([128, npc], F32)
        wti = st.tile([128, npc], I32)
        wtf = st.tile([128, npc], F32)
        wti2 = st.tile([128, npc], I32)
        wtf2 = st.tile([128, npc], F32)
        nc.vector.tensor_scalar(wx[:], vc[:, :, 0], sx, None, ALU.mult)
        nc.vector.tensor_scalar(wy[:], vc[:, :, 1], sx, None, ALU.mult)

        def frac_ip(eng, x, i_t, f_t):
            # x <- x - floor(x), robust to cast rounding mode (x >= 0)
            eng.tensor_copy(i_t[:], x)
            eng.tensor_copy(f_t[:], i_t[:])
            eng.tensor_tensor(i_t[:].bitcast(F32), f_t[:], x, ALU.is_gt)
            eng.tensor_tensor(f_t[:], f_t[:], i_t[:].bitcast(F32),
                              ALU.subtract)
            eng.tensor_tensor(x, x, f_t[:], ALU.subtract)

        frac_ip(nc.vector, wx[:], wti, wtf)
        frac_ip(nc.vector, wy[:], wti2, wtf2)
        w11 = st.tile([128, npc], F32)
        nc.vector.tensor_tensor(w11[:], wx[:], wy[:], ALU.mult)
        w01 = st.tile([128, npc], F32)
        nc.vector.tensor_tensor(w01[:], wx[:], w11[:], ALU.subtract)
        w10 = st.tile([128, npc], F32)
        nc.vector.tensor_tensor(w10[:], wy[:], w11[:], ALU.subtract)
        w00 = st.tile([128, npc], F32)
        nc.vector.tensor_scalar(wx[:], wx[:], -1.0, 1.0, ALU.mult, ALU.add)
        nc.vector.tensor_tensor(w00[:], wx[:], w10[:], ALU.subtract)


        # ---- precompute H = map_cm.T @ w1a  (pixel-major fp16 in DRAM)
        zpad = st.tile([128, 128], F16)
        nc.vector.memset(zpad[:], 0.0)
        # last FW rows of the [H[p+FW]] half have no source; zero them
        # (never gathered: idx+1 <= NPIX-FW-1)
        nc.gpsimd.dma_start(H2[NPIX - FW:NPIX, HID:2 * HID], zpad[:])
        with tc.tile_pool(name="ps_pre", bufs=2, space="PSUM") as prep, \
                tc.tile_pool(name="ps_pT", bufs=2, space="PSUM") as prepT:
            for t in range(NPIX // 512):
                mt = mpool.tile([128, 4, 512], F16)
                nc.sync.dma_start(
                    mt[:], map_cm[:, 512 * t:512 * (t + 1)]
                    .rearrange("(k p) x -> p k x", p=128))
                # stationary = w1a (4 ldweights) -> psum is [out, px]
                pho = prep.tile([128, 512], F32, tag="pho")
                for k in range(4):
                    nc.tensor.matmul(pho[:, :], w1a_sb[:, k, :], mt[:, k, :],
                                     start=(k == 0), stop=(k == 3))
                hso = hopool.tile([128, 512], F16, tag="hso")
                nc.scalar.activation(hso[:], pho[:], ACTF.Copy)
                # transpose to pixel-major [px, out] on PE
                ph = prepT.tile([128, 4, 128], F32, tag="phT")
                for sub in range(4):
                    # only the first write may start (zeroes the whole bank)
                    nc.tensor.matmul(ph[:, sub, :],
                                     hso[:, 128 * sub:128 * (sub + 1)],
                                     ident[:], start=(sub == 0),
                                     stop=(sub == 3), skip_group_check=True)
                hs = hspool.tile([128, 4, 128], F16, tag="hs")
                nc.vector.tensor_copy(hs[:], ph[:])
                nc.gpsimd.dma_start(
                    H2[512 * t:512 * (t + 1), 0:HID]
                    .rearrange("(s p) h -> p s h", p=128), hs[:])
                # second copy shifted FW rows up fills the [H[p+FW]] half
                if t == 0:
                    nc.gpsimd.dma_start(
                        H2[0:384, HID:2 * HID]
                        .rearrange("(s p) h -> p s h", p=128), hs[:, 1:4, :])
                else:
                    nc.gpsimd.dma_start(
                        H2[512 * t - FW:512 * (t + 1) - FW, HID:2 * HID]
                        .rearrange("(s p) h -> p s h", p=128), hs[:])

        # ---- main loop: gather H corners, interp, extras, ReLU, MLP2.
        # Software-pipelined: stage 2 of chunk c-1 (h1/mm2/h2/out) is
        # emitted after stage 1 of chunk c, so the in-order Act and PE
        # queues never stall on the previous chunk's tail.
        pstp = ctx.enter_context(tc.tile_pool(name="ps_t", bufs=3,
                                              space="PSUM"))
        ps2p = ctx.enter_context(tc.tile_pool(name="ps_2", bufs=3,
                                              space="PSUM"))
        def stage2(pst_p, c0_p):
            h1 = h1pool.tile([128, 512], F16, tag="h1")
            nc.scalar.activation(h1[:], pst_p[:], ACTF.Relu,
                                 bias=b1_sb[:, :])
            ps2 = ps2p.tile([128, 512], F32, tag="ps2")
            nc.tensor.matmul(ps2[:], w2_sb[:], h1[:], start=True,
                             stop=True)
            h2 = opool.tile([128, 512], F16, tag="h2")
            nc.scalar.activation(h2[:], ps2[:], ACTF.Relu,
                                 bias=b2_sb[:, :])
            nc.sync.dma_start(h_out[:, c0_p:c0_p + NCH], h2[:])

        pending = None
        for gs in range(0, ns, GCH):
            gn = min(GCH, ns - gs)
            ng = gn // 128
            g0t = gpool.tile([128, GCH // 128, 512], F16, tag="g0")
            nc.gpsimd.dma_gather(g0t[:, 0:ng, :], gsrc,
                                 idx16[:, gs // 16:(gs + gn) // 16],
                                 gn, gn, 4 * HID, elem_step=2 * HID)
            for c0 in range(gs, gs + gn, NCH):
                pst = pstp.tile([128, 512], F32, tag="pst")
                # start=True zeroes the whole 2KB PSUM bank: the bank-wide
                # extras matmul must come first, transposes accumulate after
                nc.tensor.matmul(pst[:, :], w1x_sb[:, :],
                                 ex_sb[:, c0:c0 + NCH], start=True,
                                 stop=False, skip_group_check=True)
                for g in range(4):
                    j = (c0 - gs) // 128 + g
                    col = c0 // 128 + g
                    # gathered node row: [C00, C10, C01, C11] (x-pairs of
                    # H2 rows idx, idx+1)
                    t1 = tpool.tile([128, 128], F16, tag=f"t{g}a")
                    nc.vector.tensor_scalar(
                        t1[:], g0t[:, j, 0:128], w00[:, col:col + 1],
                        None, ALU.mult)
                    nc.vector.scalar_tensor_tensor(
                        t1[:], g0t[:, j, 256:384], w01[:, col:col + 1],
                        t1[:], ALU.mult, ALU.add)
                    nc.tensor.matmul(pst[:, 128 * g:128 * (g + 1)], t1[:],
                                     ident[:], start=False, stop=False,
                                     skip_group_check=True)
                    if g == 3:
                        # last group's chainB fused on DVE (one transpose)
                        t2 = tpool.tile([128, 128], F16, tag=f"t{g}b")
                        nc.vector.tensor_scalar(
                            t2[:], g0t[:, j, 128:256], w10[:, col:col + 1],
                            None, ALU.mult)
                        nc.vector.scalar_tensor_tensor(
                            t2[:], g0t[:, j, 384:512], w11[:, col:col + 1],
                            t2[:], ALU.mult, ALU.add)
                        nc.tensor.matmul(pst[:, 128 * g:128 * (g + 1)],
                                         t2[:], ident[:], start=False,
                                         stop=True, skip_group_check=True)
                    else:
                        # chainB on Act (scaled copies); adds free in PSUM
                        t2 = tpool.tile([128, 128], F16, tag=f"t{g}b")
                        nc.scalar.activation(t2[:], g0t[:, j, 128:256],
                                             ACTF.Copy,
                                             scale=w10[:, col:col + 1])
                        t2b = tpool.tile([128, 128], F16, tag=f"t{g}c")
                        nc.scalar.activation(t2b[:], g0t[:, j, 384:512],
                                             ACTF.Copy,
                                             scale=w11[:, col:col + 1])
                        nc.tensor.matmul(pst[:, 128 * g:128 * (g + 1)],
                                         t2[:], ident[:], start=False,
                                         stop=False, skip_group_check=True)
                        nc.tensor.matmul(pst[:, 128 * g:128 * (g + 1)],
                                         t2b[:], ident[:], start=False,
                                         stop=False, skip_group_check=True)
                if pending is not None:
                    stage2(*pending)
                pending = (pst, c0)
        stage2(*pending)

    nc.compile()
    return nc


# ---------------- host side ----------------

def prep_inputs(cfg: CFG, vertices, backbone_features, seg_probs, edge_index,
                W1, W2):
    """Host prep: layout transforms + exact integer degree counts."""
    im = cfg.image_size
    v = np.asarray(vertices, np.float32)
    n = v.shape[0]
    if n < cfg.pad_n:
        v = np.concatenate([v, np.repeat(v[-1:], cfg.pad_n - n, 0)], 0)
    ep = np.asarray(edge_index).reshape(-1).astype(np.int64)
    degree = np.bincount(ep, minlength=cfg.pad_n).astype(np.float32)

    m = np.zeros((MCH, NPIX), np.float16)
    m[:480] = np.asarray(backbone_features, np.float32).reshape(480, -1)
    m[480:484] = np.asarray(seg_probs, np.float32).reshape(4, -1)
    # coords features are linear in pixel position: fold into the map
    px = np.arange(FW, dtype=np.float32) / (FW - 1)
    m[484] = np.tile(px, FH)                       # cx = ix/(W-1)
    m[485] = np.repeat(px, FW)                     # cy = iy/(H-1)

    W1 = np.asarray(W1, np.float32)
    w1a = np.zeros((MCH, 128), np.float32)
    w1a[0:480] = W1[:, 2:482].T
    w1a[480:484] = W1[:, 482:486].T
    w1a[484] = W1[:, 0]
    w1a[485] = W1[:, 1]
    w1aT = np.ascontiguousarray(w1a.reshape(4, 128, 128)).astype(np.float16)
    # degree normalization (global max) folded into the deg weight row
    w1x = np.ascontiguousarray(W1[:, 486:488].T)  # deg, dist
    w1x[0] /= degree.max() + 1e-6
    w1x = w1x.astype(np.float16)
    w2T = np.ascontiguousarray(np.asarray(W2, np.float32).T).astype(np.float16)

    # dist-to-boundary feature (host: pure elementwise data prep)
    dist = np.minimum(np.minimum(v[:, 0], im - v[:, 0]),
                      np.minimum(v[:, 1], im - v[:, 1])) / (im / 2)

    in_maps = []
    for c in range(cfg.n_cores):
        lo, hi = c * cfg.n_shard, (c + 1) * cfg.n_shard
        vcs = v[lo:hi]
        verts_w = np.ascontiguousarray(
            vcs.reshape(-1, 16, 2).transpose(2, 1, 0))       # (2,16,nwc)
        verts_w = np.ascontiguousarray(np.tile(verts_w, (1, 8, 1)))
        verts_c = np.ascontiguousarray(
            vcs.reshape(-1, 128, 2).transpose(1, 0, 2))      # (128,npc,2)
        extras = np.stack([degree[lo:hi], dist[lo:hi]]).astype(np.float16)
        in_maps.append({
            "map_cm": m, "verts_w": verts_w, "verts_c": verts_c,
            "extras": extras, "w1aT": w1aT, "w1x": w1x, "w2T": w2T,
        })
    return in_maps


_NC_CACHE: dict = {}
_NC_LOCK = threading.Lock()


def kernel(vertices, backbone_features, seg_probs, edge_index, W1, b1, W2, b2,
           image_size):
    from concourse.bass_utils import run_bass_kernel_spmd

    n = int(np.asarray(vertices).shape[0])
    n_shard = -(-n // (N_CORES * NCH)) * NCH
    cfg = CFG(n_shard, N_CORES, float(np.asarray(image_size)))

    key = (cfg.n_shard, cfg.n_cores, cfg.image_size)
    with _NC_LOCK:
        if key not in _NC_CACHE:
            _NC_CACHE[key] = build_nc(cfg)
        nc = _NC_CACHE[key]

    in_maps = prep_inputs(cfg, vertices, backbone_features, seg_probs,
                          edge_index, W1, W2)
    b1c = np.ascontiguousarray(np.asarray(b1, np.float32).reshape(128, 1))
    b2c = np.ascontiguousarray(np.asarray(b2, np.float32).reshape(128, 1))
    for im in in_maps:
        im["b1"] = b1c
        im["b2"] = b2c

    res = run_bass_kernel_spmd(nc, in_maps, core_ids=list(range(N_CORES)))
    h = np.concatenate(
        [res.results[c]["h_out"].T for c in range(N_CORES)], 0)
    return np.ascontiguousarray(h[:n]).astype(np.float32)


# revision 63
# speedup vs baseline: 1.0827x; 1.0303x over previous
"""Trainium2 Bass kernel for NodeFeatureExtractor.

Key idea: bilinear sampling is linear and so is the first MLP layer, so
they commute.  Each core precomputes H = map @ W1_mapT once (16384 px x
128 out, fp16) on the PE, then per node gathers only the 4 corner rows
of H (4 x 128 fp16 = 1KB) instead of 4 x 512 fp32 map channels (8KB).
The coords features (cx, cy) are linear in pixel position, so they fold
exactly into H as two constant map channels; only [degree, dist] remain
as a rank-2 matmul folded into the same PSUM accumulation that also
performs the interp combine + transpose (matmul against identity).

H2 DRAM row p holds [H[p], H[p+FW]] so ONE 1KB SWDGE descriptor per
node fetches all 4 corners (gathers batched 1024 idx/instruction, the
HW ucode limit).  Interp products are split DVE (scalar_tensor_tensor
chains) / Act (scaled copies); their sums are free via PSUM-accumulated
transpose-matmuls against identity, sharing the bank with the rank-2
[deg, dist] extras matmul.  Degree normalization (global max) is folded
into the extras weight on the host (counts come from host bincount
anyway — a device AllReduce acts as a global barrier in Tile and costs
~60us).  Stage 2 (ReLU, MLP2, output) of chunk c-1 is emitted after
stage 1 of chunk c so the in-order Act/PE queues pipeline cleanly.
Output is written channel-major [128, ns] fp16; host transposes.
"""
import threading
from contextlib import ExitStack

import numpy as np

import bass_rust
import concourse.bass as bass
import concourse.bacc as bacc
import concourse.mybir as mybir
import concourse.tile as tile
from concourse import masks

F32 = mybir.dt.float32
F16 = mybir.dt.float16
I32 = mybir.dt.int32
I16 = mybir.dt.int16
ALU = mybir.AluOpType
ACTF = mybir.ActivationFunctionType
AX = mybir.AxisListType

N_NODES = 200000
N_CORES = 8
HID = 128
FH = FW = 128
NPIX = FH * FW          # 16384
MCH = 512               # padded map channels (480 bb + 4 seg + 2 coord + 26 z)
NCH = 512               # nodes per compute chunk
GCH = 1024              # nodes per gather instruction (HW limit < 2048)


class CFG:
    def __init__(self, n_shard, n_cores, image_size=512.0):
        assert n_shard % NCH == 0
        self.n_shard = n_shard                      # nodes per core (padded)
        self.n_cores = n_cores
        self.pad_n = n_shard * n_cores              # padded total nodes
        self.image_size = float(image_size)


def build_nc(cfg: CFG) -> bass.Bass:
    nc = bacc.Bacc("TRN2", num_devices=cfg.n_cores)
    ns, npc = cfg.n_shard, cfg.n_shard // 128      # node cols (p-major)
    nwc = cfg.n_shard // 16                        # node cols (16-wrap)
    n_chunks = ns // NCH
    sx = (FW - 1) / cfg.image_size                 # pixel scale

    map_cm = nc.dram_tensor("map_cm", [MCH, NPIX], F16, kind="ExternalInput")
    verts_w = nc.dram_tensor("verts_w", [2, 128, nwc], F32, kind="ExternalInput")
    verts_c = nc.dram_tensor("verts_c", [128, npc, 2], F32, kind="ExternalInput")
    extras = nc.dram_tensor("extras", [2, ns], F16, kind="ExternalInput")
    w1aT = nc.dram_tensor("w1aT", [4, 128, 128], F16, kind="ExternalInput")
    w1x = nc.dram_tensor("w1x", [2, 128], F16, kind="ExternalInput")
    w2T = nc.dram_tensor("w2T", [128, 128], F16, kind="ExternalInput")
    b1 = nc.dram_tensor("b1", [128, 1], F32, kind="ExternalInput")
    b2 = nc.dram_tensor("b2", [128, 1], F32, kind="ExternalInput")
    h_out = nc.dram_tensor("h_out", [128, ns], F16, kind="ExternalOutput")
    # H2 row p holds [H[p], H[p+FW]]: one 1KB gather descriptor covering
    # rows p..p+1 delivers all 4 bilinear corners of a node
    H2 = nc.dram_tensor("H2", [NPIX, 2 * HID], F16, kind="Internal")
    gsrc = bass_rust.AP(H2[:, :].tensor, 0,
                        [[2 * HID, NPIX - 2], [1, 4 * HID]])

    with tile.TileContext(nc) as tc, ExitStack() as ctx:

        st = ctx.enter_context(tc.tile_pool(name="static", bufs=1))
        dram = ctx.enter_context(tc.tile_pool(name="dram", bufs=1, space="DRAM"))
        mpool = ctx.enter_context(tc.tile_pool(name="mapp", bufs=4))
        hopool = ctx.enter_context(tc.tile_pool(name="hop", bufs=3))
        hspool = ctx.enter_context(tc.tile_pool(name="hsp", bufs=3))
        gpool = ctx.enter_context(tc.tile_pool(name="gather", bufs=4))
        tpool = ctx.enter_context(tc.tile_pool(name="tmps", bufs=2))
        h1pool = ctx.enter_context(tc.tile_pool(name="h1p", bufs=2))
        opool = ctx.enter_context(tc.tile_pool(name="outs", bufs=2))


        ident = st.tile([128, 128], F16)
        masks.make_identity(nc, ident[:])

        # ---- static loads
        w1a_sb = st.tile([128, 4, 128], F16)
        nc.sync.dma_start(w1a_sb[:], w1aT[:, :, :].rearrange("k p m -> p k m"))
        w1x_sb = st.tile([2, 128], F16)
        nc.sync.dma_start(w1x_sb[:], w1x[:, :])
        w2_sb = st.tile([128, 128], F16)
        nc.sync.dma_start(w2_sb[:], w2T[:, :])
        b1_sb = st.tile([128, 1], F32)
        nc.sync.dma_start(b1_sb[:], b1[:, :])
        b2_sb = st.tile([128, 1], F32)
        nc.sync.dma_start(b2_sb[:], b2[:, :])
        ex_sb = st.tile([2, ns], F16)
        nc.sync.dma_start(ex_sb[:], extras[:, :])

        # ---- batched gather-index computation (16-wrap layout)
        vw = st.tile([128, 2, nwc], F32)
        nc.sync.dma_start(vw[:], verts_w[:, :, :].rearrange("d p c -> p d c"))
        fx = st.tile([128, nwc], F32)
        fy = st.tile([128, nwc], F32)
        ti = st.tile([128, nwc], I32)
        tf = st.tile([128, nwc], F32)
        ti2 = st.tile([128, nwc], I32)
        tf2 = st.tile([128, nwc], F32)

        def floor_ip(eng, x, i_t, f_t):
            # x <- floor(x), robust to cast rounding mode (x >= 0)
            eng.tensor_copy(i_t[:], x)
            eng.tensor_copy(f_t[:], i_t[:])
            eng.tensor_tensor(x, f_t[:], x, ALU.is_gt)   # x = (f > x)
            eng.tensor_tensor(x, f_t[:], x, ALU.subtract)

        nc.vector.tensor_scalar(fx[:], vw[:, 0, :], sx, None, ALU.mult)
        floor_ip(nc.vector, fx[:], ti, tf)
        nc.vector.tensor_scalar(fy[:], vw[:, 1, :], sx, None, ALU.mult)
        floor_ip(nc.vector, fy[:], ti2, tf2)
        nc.vector.scalar_tensor_tensor(fx[:], fy[:], float(FW), fx[:],
                                       ALU.mult, ALU.add)
        idx16 = st.tile([128, nwc], I16)
        nc.vector.tensor_copy(idx16[:], fx[:])

        # ---- per-node bilinear weights (p-major layout)
        vc = st.tile([128, npc, 2], F32)
        nc.sync.dma_start(vc[:], verts_c[:, :, :])
        wx = st.tile([128, npc], F32)
        wy = st.tile([128, npc], F32)
        wti = st.tile([128, npc], I32)
        wtf = st.tile([128, npc], F32)
        wti2 = st.tile([128, npc], I32)
        wtf2 = st.tile([128, npc], F32)
        nc.vector.tensor_scalar(wx[:], vc[:, :, 0], sx, None, ALU.mult)
        nc.vector.tensor_scalar(wy[:], vc[:, :, 1], sx, None, ALU.mult)

        def frac_ip(eng, x, i_t, f_t):
            # x <- x - floor(x), robust to cast rounding mode (x >= 0)
            eng.tensor_copy(i_t[:], x)
            eng.tensor_copy(f_t[:], i_t[:])
            eng.tensor_tensor(i_t[:].bitcast(F32), f_t[:], x, ALU.is_gt)
            eng.tensor_tensor(f_t[:], f_t[:], i_t[:].bitcast(F32),
                              ALU.subtract)
            eng.tensor_tensor(x, x, f_t[:], ALU.subtract)

        frac_ip(nc.vector, wx[:], wti, wtf)
        frac_ip(nc.vector, wy[:], wti2, wtf2)
        w11 = st.tile([128, npc], F32)
        nc.vector.tensor_tensor(w11[:], wx[:], wy[:], ALU.mult)
        w01 = st.tile([128, npc], F32)
        nc.vector.tensor_tensor(w01[:], wx[:], w11[:], ALU.subtract)
        w10 = st.tile([128, npc], F32)
        nc.vector.tensor_tensor(w10[:], wy[:], w11[:], ALU.subtract)
        w00 = st.tile([128, npc], F32)
        nc.vector.tensor_scalar(wx[:], wx[:], -1.0, 1.0, ALU.mult, ALU.add)
        nc.vector.tensor_tensor(w00[:], wx[:], w10[:], ALU.subtract)


        # ---- precompute H = map_cm.T @ w1a  (pixel-major fp16 in DRAM)
        zpad = st.tile([128, 128], F16)
        nc.vector.memset(zpad[:], 0.0)
        # last FW rows of the [H[p+FW]] half have no source; zero them
        # (never gathered: idx+1 <= NPIX-FW-1)
        nc.gpsimd.dma_start(H2[NPIX - FW:NPIX, HID:2 * HID], zpad[:])
        with tc.tile_pool(name="ps_pre", bufs=2, space="PSUM") as prep, \
                tc.tile_pool(name="ps_pT", bufs=2, space="PSUM") as prepT:
            for t in range(NPIX // 512):
                mt = mpool.tile([128, 4, 512], F16)
                nc.sync.dma_start(
                    mt[:], map_cm[:, 512 * t:512 * (t + 1)]
                    .rearrange("(k p) x -> p k x", p=128))
                # stationary = w1a (4 ldweights) -> psum is [out, px]
                pho = prep.tile([128, 512], F32, tag="pho")
                for k in range(4):
                    nc.tensor.matmul(pho[:, :], w1a_sb[:, k, :], mt[:, k, :],
                                     start=(k == 0), stop=(k == 3))
                hso = hopool.tile([128, 512], F16, tag="hso")
                nc.scalar.activation(hso[:], pho[:], ACTF.Copy)
                # transpose to pixel-major [px, out] on PE
                ph = prepT.tile([128, 4, 128], F32, tag="phT")
                for sub in range(4):
                    # only the first write may start (zeroes the whole bank)
                    nc.tensor.matmul(ph[:, sub, :],
                                     hso[:, 128 * sub:128 * (sub + 1)],
                                     ident[:], start=(sub == 0),
                                     stop=(sub == 3), skip_group_check=True)
                hs = hspool.tile([128, 4, 128], F16, tag="hs")
                nc.scalar.activation(hs[:], ph[:], ACTF.Copy)
                nc.gpsimd.dma_start(
                    H2[512 * t:512 * (t + 1), 0:HID]
                    .rearrange("(s p) h -> p s h", p=128), hs[:])
                # second copy shifted FW rows up fills the [H[p+FW]] half
                if t == 0:
                    nc.gpsimd.dma_start(
                        H2[0:384, HID:2 * HID]
                        .rearrange("(s p) h -> p s h", p=128), hs[:, 1:4, :])
                else:
                    nc.gpsimd.dma_start(
                        H2[512 * t - FW:512 * (t + 1) - FW, HID:2 * HID]
                        .rearrange("(s p) h -> p s h", p=128), hs[:])

        # ---- main loop: gather H corners, interp, extras, ReLU, MLP2.
        # Software-pipelined: stage 2 of chunk c-1 (h1/mm2/h2/out) is
        # emitted after stage 1 of chunk c, so the in-order Act and PE
        # queues never stall on the previous chunk's tail.
        pstp = ctx.enter_context(tc.tile_pool(name="ps_t", bufs=3,
                                              space="PSUM"))
        ps2p = ctx.enter_context(tc.tile_pool(name="ps_2", bufs=3,
                                              space="PSUM"))
        def stage2(pst_p, c0_p):
            h1 = h1pool.tile([128, 512], F16, tag="h1")
            nc.scalar.activation(h1[:], pst_p[:], ACTF.Relu,
                                 bias=b1_sb[:, :])
            ps2 = ps2p.tile([128, 512], F32, tag="ps2")
            nc.tensor.matmul(ps2[:], w2_sb[:], h1[:], start=True,
                             stop=True)
            h2 = opool.tile([128, 512], F16, tag="h2")
            nc.scalar.activation(h2[:], ps2[:], ACTF.Relu,
                                 bias=b2_sb[:, :])
            nc.sync.dma_start(h_out[:, c0_p:c0_p + NCH], h2[:])

        pending = None
        for gs in range(0, ns, GCH):
            gn = min(GCH, ns - gs)
            ng = gn // 128
            g0t = gpool.tile([128, GCH // 128, 512], F16, tag="g0")
            nc.gpsimd.dma_gather(g0t[:, 0:ng, :], gsrc,
                                 idx16[:, gs // 16:(gs + gn) // 16],
                                 gn, gn, 4 * HID, elem_step=2 * HID)
            for c0 in range(gs, gs + gn, NCH):
                pst = pstp.tile([128, 512], F32, tag="pst")
                # start=True zeroes the whole 2KB PSUM bank: the bank-wide
                # extras matmul must come first, transposes accumulate after
                nc.tensor.matmul(pst[:, :], w1x_sb[:, :],
                                 ex_sb[:, c0:c0 + NCH], start=True,
                                 stop=False, skip_group_check=True)
                for g in range(4):
                    j = (c0 - gs) // 128 + g
                    col = c0 // 128 + g
                    # gathered node row: [C00, C10, C01, C11] (x-pairs of
                    # H2 rows idx, idx+1)
                    t1 = tpool.tile([128, 128], F16, tag=f"t{g}a")
                    nc.vector.tensor_scalar(
                        t1[:], g0t[:, j, 0:128], w00[:, col:col + 1],
                        None, ALU.mult)
                    nc.vector.scalar_tensor_tensor(
                        t1[:], g0t[:, j, 256:384], w01[:, col:col + 1],
                        t1[:], ALU.mult, ALU.add)
                    nc.tensor.matmul(pst[:, 128 * g:128 * (g + 1)], t1[:],
                                     ident[:], start=False, stop=False,
                                     skip_group_check=True)
                    if g == 3:
                        # last group's chainB fused on DVE (one transpose)
                        t2 = tpool.tile([128, 128], F16, tag=f"t{g}b")
                        nc.vector.tensor_scalar(
                            t2[:], g0t[:, j, 128:256], w10[:, col:col + 1],
                            None, ALU.mult)
                        nc.vector.scalar_tensor_tensor(
                            t2[:], g0t[:, j, 384:512], w11[:, col:col + 1],
                            t2[:], ALU.mult, ALU.add)
                        nc.tensor.matmul(pst[:, 128 * g:128 * (g + 1)],
                                         t2[:], ident[:], start=False,
                                         stop=True, skip_group_check=True)
                    else:
                        # chainB on Act (scaled copies); adds free in PSUM
                        t2 = tpool.tile([128, 128], F16, tag=f"t{g}b")
                        nc.scalar.activation(t2[:], g0t[:, j, 128:256],
                                             ACTF.Copy,
                                             scale=w10[:, col:col + 1])
                        t2b = tpool.tile([128, 128], F16, tag=f"t{g}c")
                        nc.scalar.activation(t2b[:], g0t[:, j, 384:512],
                                             ACTF.Copy,
                                             scale=w11[:, col:col + 1])
                        nc.tensor.matmul(pst[:, 128 * g:128 * (g + 1)],
                                         t2[:], ident[:], start=False,
                                         stop=False, skip_group_check=True)
                        nc.tensor.matmul(pst[:, 128 * g:128 * (g + 1)],
                                         t2b[:], ident[:], start=False,
                                         stop=False, skip_group_check=True)
                if pending is not None:
                    stage2(*pending)
                pending = (pst, c0)
        stage2(*pending)

    nc.compile()
    return nc


# ---------------- host side ----------------

def prep_inputs(cfg: CFG, vertices, backbone_features, seg_probs, edge_index,
                W1, W2):
    """Host prep: layout transforms + exact integer degree counts."""
    im = cfg.image_size
    v = np.asarray(vertices, np.float32)
    n = v.shape[0]
    if n < cfg.pad_n:
        v = np.concatenate([v, np.repeat(v[-1:], cfg.pad_n - n, 0)], 0)
    ep = np.asarray(edge_index).reshape(-1).astype(np.int64)
    degree = np.bincount(ep, minlength=cfg.pad_n).astype(np.float32)

    m = np.zeros((MCH, NPIX), np.float16)
    m[:480] = np.asarray(backbone_features, np.float32).reshape(480, -1)
    m[480:484] = np.asarray(seg_probs, np.float32).reshape(4, -1)
    # coords features are linear in pixel position: fold into the map
    px = np.arange(FW, dtype=np.float32) / (FW - 1)
    m[484] = np.tile(px, FH)                       # cx = ix/(W-1)
    m[485] = np.repeat(px, FW)                     # cy = iy/(H-1)

    W1 = np.asarray(W1, np.float32)
    w1a = np.zeros((MCH, 128), np.float32)
    w1a[0:480] = W1[:, 2:482].T
    w1a[480:484] = W1[:, 482:486].T
    w1a[484] = W1[:, 0]
    w1a[485] = W1[:, 1]
    w1aT = np.ascontiguousarray(w1a.reshape(4, 128, 128)).astype(np.float16)
    # degree normalization (global max) folded into the deg weight row
    w1x = np.ascontiguousarray(W1[:, 486:488].T)  # deg, dist
    w1x[0] /= degree.max() + 1e-6
    w1x = w1x.astype(np.float16)
    w2T = np.ascontiguousarray(np.asarray(W2, np.float32).T).astype(np.float16)

    # dist-to-boundary feature (host: pure elementwise data prep)
    dist = np.minimum(np.minimum(v[:, 0], im - v[:, 0]),
                      np.minimum(v[:, 1], im - v[:, 1])) / (im / 2)

    in_maps = []
    for c in range(cfg.n_cores):
        lo, hi = c * cfg.n_shard, (c + 1) * cfg.n_shard
        vcs = v[lo:hi]
        verts_w = np.ascontiguousarray(
            vcs.reshape(-1, 16, 2).transpose(2, 1, 0))       # (2,16,nwc)
        verts_w = np.ascontiguousarray(np.tile(verts_w, (1, 8, 1)))
        verts_c = np.ascontiguousarray(
            vcs.reshape(-1, 128, 2).transpose(1, 0, 2))      # (128,npc,2)
        extras = np.stack([degree[lo:hi], dist[lo:hi]]).astype(np.float16)
        in_maps.append({
            "map_cm": m, "verts_w": verts_w, "verts_c": verts_c,
            "extras": extras, "w1aT": w1aT, "w1x": w1x, "w2T": w2T,
        })
    return in_maps


_NC_CACHE: dict = {}
_NC_LOCK = threading.Lock()


def kernel(vertices, backbone_features, seg_probs, edge_index, W1, b1, W2, b2,
           image_size):
    from concourse.bass_utils import run_bass_kernel_spmd

    n = int(np.asarray(vertices).shape[0])
    n_shard = -(-n // (N_CORES * NCH)) * NCH
    cfg = CFG(n_shard, N_CORES, float(np.asarray(image_size)))

    key = (cfg.n_shard, cfg.n_cores, cfg.image_size)
    with _NC_LOCK:
        if key not in _NC_CACHE:
            _NC_CACHE[key] = build_nc(cfg)
        nc = _NC_CACHE[key]

    in_maps = prep_inputs(cfg, vertices, backbone_features, seg_probs,
                          edge_index, W1, W2)
    b1c = np.ascontiguousarray(np.asarray(b1, np.float32).reshape(128, 1))
    b2c = np.ascontiguousarray(np.asarray(b2, np.float32).reshape(128, 1))
    for im in in_maps:
        im["b1"] = b1c
        im["b2"] = b2c

    res = run_bass_kernel_spmd(nc, in_maps, core_ids=list(range(N_CORES)))
    h = np.concatenate(
        [res.results[c]["h_out"].T for c in range(N_CORES)], 0)
    return np.ascontiguousarray(h[:n]).astype(np.float32)
